# revision 36
# baseline (speedup 1.0000x reference)
"""Trainium2 Bass kernel for nn_Predictor_67585605370461 (segment_reduce).

Per patch (N=4194304, 9 elements each):
  S = sign(E-0.5) in {-1,+1}; g = sum(S); sall = sum(V); sds = sum(V*S)
  out01 = (9*sds <= g*sall)   [equivalent to md <= ma]
  unknown m = (|g| == 9); b = o01 (o01==1 for unknowns)
Global masked means avgB/avgW of center pixels over the two known classes;
  corr = o01 - m*(w1 < 0), w1 = v*s - mid*s, s = avgW-avgB, mid = (avgB+avgW)/2

Engine split (all big 9-wide group reductions on the otherwise-idle PE via
identity-matmul PSUM accumulation in fp32r; ~2^-14 rounding, validated):
  Act:  Vr = round_f32r(V), S = sign(E-0.5) as f32r
  DVE:  U = Vr*S (f32r), center extract, classification smalls + stat accums
  PE:   sall/sds/g = 9-slice Id-matmul accumulations (fp32r, 1 cyc/col)
  Pool: phase-2 helper chunks
Cross-core: one [1,5] -> [8,5] AllGather of per-core stat partials (cheaper
than AllReduce), then a tiny PE ones-matmul partition reduce.

Sharding: data-parallel over the patch axis, 524288 patches per core.
"""

import numpy as np

import concourse.bass as bass
import concourse.bacc as bacc
import concourse.mybir as mybir
import concourse.tile as tile
from concourse import bass_utils

N_CORES = 8
N_PATCH = 4194304
NP_CORE = N_PATCH // N_CORES  # 524288
P = 128
NINE = 9
NPW = NP_CORE // P           # 4096 patches per partition per core
TILES = [256] * 15 + [128, 96, 32]   # sum = 4096
NT = len(TILES)
NDVE = 0          # trailing tiles on the exact all-DVE path (short drain)
WMAX = max(TILES)
H_OUT = 2048
PH2_SIZES = [512] * 8

f32 = mybir.dt.float32
f32r = mybir.dt.float32r
bf16 = mybir.dt.bfloat16
Alu = mybir.AluOpType
Act = mybir.ActivationFunctionType
X = mybir.AxisListType.X

_CACHE = {}


def _build(stub_cc=False):
    num_devices = 1 if stub_cc else N_CORES
    nc = bacc.Bacc("TRN2", target_bir_lowering=False, debug=False,
                   num_devices=num_devices)
    img = nc.dram_tensor("img", [NP_CORE, NINE], f32r, kind="ExternalInput")
    edg = nc.dram_tensor("edg", [NP_CORE, NINE], f32, kind="ExternalInput")
    idd = nc.dram_tensor("ident", [P, P], f32, kind="ExternalInput")
    out = nc.dram_tensor("out", [NP_CORE], f32, kind="ExternalOutput")

    # partition-contiguous patch layout: partition p owns patches
    # [p*NPW, (p+1)*NPW) of this core's shard; column slices select tiles.
    img_r = img.ap().rearrange("(p j) n -> p (j n)", p=P)
    edg_r = edg.ap().rearrange("(p j) n -> p (j n)", p=P)
    out_f = out.ap().rearrange("(p j) -> p j", p=P)

    with tile.TileContext(nc) as tc:
        with (
            tc.tile_pool(name="vin", bufs=5) as vpool,
            tc.tile_pool(name="ein", bufs=5) as epool,
            tc.tile_pool(name="sr", bufs=5) as spool,
            tc.tile_pool(name="sm", bufs=2) as smpool,
            tc.tile_pool(name="ph2", bufs=3) as ph2pool,
            tc.tile_pool(name="persist", bufs=1) as pers,
            tc.tile_pool(name="psum", bufs=3, space="PSUM") as psum,
            tc.tile_pool(name="psmall", bufs=1, space="PSUM") as psmall,
            tc.tile_pool(name="dram", bufs=1, space="DRAM") as dram,
        ):
            vals = pers.tile([P, NPW], f32)
            marr = pers.tile([P, NPW], bf16)
            barr = pers.tile([P, NPW], bf16)
            # per-tile accumulator columns: [SO | SU | CO | CU | SA] x NT
            acc = pers.tile([P, 5 * NT], f32)
            accSO = acc[:, 0 * NT:1 * NT]
            accSU = acc[:, 1 * NT:2 * NT]
            accCO = acc[:, 2 * NT:3 * NT]
            accCU = acc[:, 3 * NT:4 * NT]
            accSA = acc[:, 4 * NT:5 * NT]
            nbias = pers.tile([P, 1], f32)
            nc.vector.memset(nbias[:], -0.5)
            ones_col = pers.tile([P, 1], f32)
            nc.vector.memset(ones_col[:], 1.0)
            ones8 = pers.tile([8, P], f32)
            nc.vector.memset(ones8[:], 1.0)
            scbP = pers.tile([P, 2], f32)
            scr = pers.tile([P, WMAX], f32)

            # identity: f32r for the V/U reduces, bf16 for the S (sign) reduce
            idf = pers.tile([P, P], f32)
            nc.sync.dma_start(idf[:], idd.ap())
            idr = pers.tile([P, P], f32r)
            nc.scalar.activation(idr[:], idf[:], Act.Identity)
            idb = pers.tile([P, P], bf16)
            nc.scalar.activation(idb[:], idf[:], Act.Identity)

            offs = [sum(TILES[:i]) for i in range(NT)]
            st = {}   # per-tile live state

            def stage_load(i):
                w = TILES[i]
                F = w * NINE
                o9 = offs[i] * NINE
                Vt = vpool.tile([P, WMAX * NINE], f32r, tag="V")
                nc.sync.dma_start(Vt[:, :F], img_r[:, o9:o9 + F])
                Et = epool.tile([P, WMAX * NINE], f32, tag="E")
                nc.sync.dma_start(Et[:, :F], edg_r[:, o9:o9 + F])
                st[i] = {"V": Vt, "E": Et}

            def stage_a(i):
                # Act: S = sign(E-0.5) as bf16 (exact); DVE: center extract
                w = TILES[i]
                F = w * NINE
                d = st[i]
                St = spool.tile([P, WMAX * NINE], bf16, tag="S")
                nc.scalar.activation(St[:, :F], d["E"][:, :F], Act.Sign,
                                     bias=nbias[:])
                v3 = d["V"][:, :F].rearrange("p (w n) -> p w n", n=NINE)
                vsl = vals[:, offs[i]:offs[i] + w]
                nc.vector.tensor_scalar(vsl, v3[:, :, 4], 1.0, None,
                                        op0=Alu.mult, op1=Alu.add,
                                        accum_out=accSA[:, i:i + 1])
                d["S"] = St

            def stage_b(i):
                # PE: sall/g group reduces
                w = TILES[i]
                F = w * NINE
                d = st[i]
                ps = psum.tile([P, 2 * 512], f32, tag="ps")
                V3 = d["V"][:, :F].rearrange("p (w n) -> p w n", n=NINE)
                S3 = d["S"][:, :F].rearrange("p (w n) -> p w n", n=NINE)
                for j in range(NINE):
                    nc.tensor.matmul(ps[:, 0:w], idr[:], V3[:, :, j],
                                     start=(j == 0), stop=(j == NINE - 1))
                for j in range(NINE):
                    nc.tensor.matmul(ps[:, 512:512 + w], idb[:],
                                     S3[:, :, j],
                                     start=(j == 0), stop=(j == NINE - 1))
                d.update(sall=ps[:, 0:w], sds=ps[:, 256:256 + w],
                         g=ps[:, 512:512 + w], ps=ps)

            def stage_c(i):
                # U = V*S in place (split DVE/Pool), then PE sds group reduce
                w = TILES[i]
                F = w * NINE
                d = st[i]
                Vt, St = d["V"], d["S"]
                if w >= 256:
                    H = (F // 2) // 2 * 2
                    nc.vector.tensor_tensor(Vt[:, :H], Vt[:, :H], St[:, :H],
                                            op=Alu.mult)
                    nc.gpsimd.tensor_tensor(Vt[:, H:F], Vt[:, H:F],
                                            St[:, H:F], op=Alu.mult)
                else:
                    nc.vector.tensor_tensor(Vt[:, :F], Vt[:, :F], St[:, :F],
                                            op=Alu.mult)
                U3 = Vt[:, :F].rearrange("p (w n) -> p w n", n=NINE)
                ps = d["ps"]
                for j in range(NINE):
                    nc.tensor.matmul(ps[:, 256:256 + w], idr[:],
                                     U3[:, :, j],
                                     start=(j == 0), stop=(j == NINE - 1))

            def stage_d(i):
                # DVE: classification smalls + stat accums
                w = TILES[i]
                d = st[i]
                sall, sds, g = d["sall"], d["sds"], d["g"]
                vsl = vals[:, offs[i]:offs[i] + w]
                msl = marr[:, offs[i]:offs[i] + w]
                bsl = barr[:, offs[i]:offs[i] + w]
                # stage g into SBUF (verifier: only one PSUM read per op);
                # gsq = g^2 for the unknown test (g^2 == 81 <=> |g| == 9)
                gsb = smpool.tile([P, WMAX], f32, tag="gsb")
                nc.scalar.activation(gsb[:, :w], g, Act.Identity)
                gsq = smpool.tile([P, WMAX], f32, tag="gsq")
                nc.scalar.activation(gsq[:, :w], g, Act.Square)
                q = smpool.tile([P, WMAX], f32, tag="q")
                nc.vector.tensor_tensor(q[:, :w], gsb[:, :w], sall,
                                        op=Alu.mult)
                # o01 = (9*sds <= g*sall)
                nc.vector.scalar_tensor_tensor(bsl, sds, 9.0, q[:, :w],
                                               op0=Alu.mult, op1=Alu.is_le)
                # m = (g^2 == 81); CU accum
                nc.vector.tensor_scalar(msl, gsq[:, :w], 81.0, None,
                                        op0=Alu.is_equal, op1=Alu.add,
                                        accum_out=accCU[:, i:i + 1])
                # A = o01 - m/2 in place over bsl (phase 2: corr = A+m*sg/2)
                # accum gives CO' = CO - CU/2
                nc.vector.scalar_tensor_tensor(bsl, msl, -0.5, bsl,
                                               op0=Alu.mult, op1=Alu.add,
                                               accum_out=accCO[:, i:i + 1])
                # masked center sums (outputs are scratch)
                # SO' = sum(A*v) = SO - SU/2
                nc.vector.scalar_tensor_tensor(scr[:, :w], bsl, 1.0, vsl,
                                               op0=Alu.mult, op1=Alu.mult,
                                               accum_out=accSO[:, i:i + 1])
                nc.vector.scalar_tensor_tensor(scr[:, :w], msl, 1.0, vsl,
                                               op0=Alu.mult, op1=Alu.mult,
                                               accum_out=accSU[:, i:i + 1])
                del st[i]

            # Skewed software pipeline (c before a/b keeps Pool's U-half of
            # tile k-2 ahead of the k-1 rounding copy in the Pool queue).
            for k in range(NT + 3):
                if 0 <= k - 3 < NT:
                    stage_d(k - 3)
                if k < NT:
                    stage_load(k)
                if 0 <= k - 2 < NT:
                    stage_c(k - 2)
                if 0 <= k - 1 < NT:
                    stage_a(k - 1)
                    stage_b(k - 1)

            # ---- per-core stats -> [1,5] ----
            acc8 = pers.tile([P, 5], f32)
            acc5 = acc[:].rearrange("p (j i) -> p j i", j=5)
            nc.vector.tensor_reduce(acc8[:], acc5, axis=X, op=Alu.add)
            g1 = psmall.tile([1, 8 * 5], f32)
            nc.tensor.matmul(g1[:, 0:5], ones_col[:], acc8[:], start=True,
                             stop=True)
            gsb = pers.tile([1, 5], f32)
            nc.vector.tensor_copy(gsb[:], g1[:, 0:5])

            # ---- AllGather the [1,5] partials across cores ----
            cc_in = dram.tile([1, 5], f32)
            cc_out = dram.tile([8, 5], f32, addr_space="Shared")
            nc.sync.dma_start(cc_in[:], gsb[:])
            gt8 = pers.tile([8, 5], f32)
            if stub_cc:
                nc.vector.memset(gt8[:], 0.0)
                nc.sync.dma_start(gt8[0:1, :], cc_in[:])
            else:
                nc.gpsimd.collective_compute(
                    "AllGather", Alu.bypass,
                    replica_groups=[list(range(N_CORES))],
                    ins=[cc_in[:].opt()], outs=[cc_out[:].opt()])
                nc.sync.dma_start(gt8[:], cc_out[:])
            # sum the 8 per-core rows AND broadcast to all partitions in one
            # matmul: ones[8,P].T @ gt8[8,5] -> [P,5]
            pb = psmall.tile([P, 8], f32)
            nc.tensor.matmul(pb[:, 0:5], ones8[:], gt8[:], start=True,
                             stop=True)
            gtP = pers.tile([P, 5], f32)
            nc.vector.tensor_copy(gtP[:], pb[:, 0:5])

            # gtP cols = [SO', SU, CO', CU, SA], SO' = SO-SU/2, CO' = CO-CU/2
            # SW = SO-SU = SO'-SU/2, CW = CO-CU = CO'-CU/2,
            # SB = SA-SO = SA-SO'-SU/2, CB = N-CO
            tmp = pers.tile([P, 8], f32)
            avg = pers.tile([P, 2], f32)
            rc = pers.tile([P, 2], f32)
            nc.vector.scalar_tensor_tensor(tmp[:, 5:6], gtP[:, 3:4], -0.5,
                                           gtP[:, 2:3], op0=Alu.mult,
                                           op1=Alu.add)          # CW
            nc.vector.scalar_tensor_tensor(tmp[:, 2:3], gtP[:, 3:4], 0.5,
                                           gtP[:, 2:3], op0=Alu.mult,
                                           op1=Alu.add)          # CO
            nc.vector.tensor_scalar(tmp[:, 4:5], tmp[:, 2:3], -1.0,
                                    float(N_PATCH), op0=Alu.mult,
                                    op1=Alu.add)                 # CB
            nc.vector.reciprocal(rc[:], tmp[:, 4:6])             # [rcb, rcw]
            nc.vector.scalar_tensor_tensor(tmp[:, 1:2], gtP[:, 1:2], -0.5,
                                           gtP[:, 0:1], op0=Alu.mult,
                                           op1=Alu.add)          # SW
            nc.vector.tensor_tensor(tmp[:, 7:8], gtP[:, 4:5], gtP[:, 0:1],
                                    op=Alu.subtract)             # SA-SO'
            nc.vector.scalar_tensor_tensor(tmp[:, 0:1], gtP[:, 1:2], -0.5,
                                           tmp[:, 7:8], op0=Alu.mult,
                                           op1=Alu.add)          # SB
            nc.vector.tensor_tensor(avg[:], tmp[:, 0:2], rc[:],
                                    op=Alu.mult)                 # [avgB, avgW]
            nc.vector.tensor_tensor(scbP[:, 0:1], avg[:, 1:2], avg[:, 0:1],
                                    op=Alu.subtract)             # s
            nc.vector.tensor_tensor(tmp[:, 6:7], avg[:, 0:1], avg[:, 1:2],
                                    op=Alu.add)                  # avgB+avgW
            nc.vector.scalar_tensor_tensor(scbP[:, 1:2], tmp[:, 6:7], -0.5,
                                           scbP[:, 0:1], op0=Alu.mult,
                                           op1=Alu.mult)         # -mid*s

            # ---- phase 2: corr = A + 0.5*m*sign(v*s - mid*s) ----
            # (A = o01 - m/2 is in barr; sign==-1 exactly when w1<0)
            PH2MAX = max(PH2_SIZES)
            po = 0
            for ci, cw in enumerate(PH2_SIZES):
                sl = slice(po, po + cw)
                sg = ph2pool.tile([P, PH2MAX], bf16, tag="sg")
                nc.scalar.activation(sg[:, :cw], vals[:, sl], Act.Sign,
                                     bias=scbP[:, 1:2], scale=scbP[:, 0:1])
                h2 = ph2pool.tile([P, PH2MAX], bf16, tag="h2")
                nc.vector.scalar_tensor_tensor(h2[:, :cw], marr[:, sl], 0.5,
                                               sg[:, :cw], op0=Alu.mult,
                                               op1=Alu.mult)
                corr = ph2pool.tile([P, PH2MAX], f32, tag="corr")
                nc.vector.tensor_tensor(corr[:, :cw], barr[:, sl],
                                        h2[:, :cw], op=Alu.add)
                nc.sync.dma_start(out_f[:, sl], corr[:, :cw])
                po += cw

    nc.compile()
    return nc


def _get_nc():
    if "nc" not in _CACHE:
        _CACHE["nc"] = _build()
    return _CACHE["nc"]


def run(image, edges_prob, gt=None, trace=False, tmpdir=None):
    nc = _get_nc()
    img = np.ascontiguousarray(np.asarray(image), dtype=np.float32)
    edg = np.ascontiguousarray(np.asarray(edges_prob), dtype=np.float32)
    img = img.reshape(N_PATCH, NINE)
    edg = edg.reshape(N_PATCH, NINE)
    ident = np.eye(P, dtype=np.float32)
    in_maps = []
    for c in range(N_CORES):
        sl = slice(c * NP_CORE, (c + 1) * NP_CORE)
        in_maps.append({"img": img[sl], "edg": edg[sl], "ident": ident})
    res = bass_utils.run_bass_kernel_spmd(
        nc, in_maps, core_ids=list(range(N_CORES)),
        trace=trace, tmpdir=tmpdir)
    shards = []
    for c in range(N_CORES):
        shards.append(res.results[c]["out"])
    full = np.concatenate(shards).reshape(H_OUT, H_OUT)
    return full, res


def kernel(image, edges_prob, gt=None, **_ignored):
    full, _ = run(image, edges_prob, gt)
    return full


def _numpy_model(image, edges_prob):
    img = np.asarray(image).reshape(N_PATCH, NINE)
    edg = np.asarray(edges_prob).reshape(N_PATCH, NINE)
    S = np.where(edg > 0.5, 1.0, -1.0).astype(np.float32)
    g = S.sum(1)
    sds = (img * S).sum(1)
    sall = img.sum(1)
    o01 = (9.0 * sds <= g * sall).astype(np.float32)
    unk = np.abs(g) == 9
    v = img[:, 4]
    mb = (~unk) & (o01 == 0.0)
    mw = (~unk) & (o01 == 1.0)
    avgB = (v * mb).sum() / max(mb.sum(), 1)
    avgW = (v * mw).sum() / max(mw.sum(), 1)
    cls = (np.abs(v - avgB) >= np.abs(v - avgW)).astype(np.float32)
    corr = np.where(unk, cls, o01)
    return corr.reshape(H_OUT, H_OUT)


# revision 42
# speedup vs baseline: 1.0354x; 1.0354x over previous
"""Trainium2 Bass kernel for nn_Predictor_67585605370461 (segment_reduce).

Per patch (N=4194304, 9 elements each):
  S = sign(E-0.5) in {-1,+1}; g = sum(S); sall = sum(V); sds = sum(V*S)
  out01 = (9*sds <= g*sall)   [equivalent to md <= ma]
  unknown m = (|g| == 9); b = o01 (o01==1 for unknowns)
Global masked means avgB/avgW of center pixels over the two known classes;
  corr = o01 - m*(w1 < 0), w1 = v*s - mid*s, s = avgW-avgB, mid = (avgB+avgW)/2

Engine split (all big 9-wide group reductions on the otherwise-idle PE via
identity-matmul PSUM accumulation in fp32r; ~2^-14 rounding, validated):
  Act:  Vr = round_f32r(V), S = sign(E-0.5) as f32r
  DVE:  U = Vr*S (f32r), center extract, classification smalls + stat accums
  PE:   sall/sds/g = 9-slice Id-matmul accumulations (fp32r, 1 cyc/col)
  Pool: phase-2 helper chunks
Cross-core: one [1,5] -> [8,5] AllGather of per-core stat partials (cheaper
than AllReduce), then a tiny PE ones-matmul partition reduce.

Sharding: data-parallel over the patch axis, 524288 patches per core.
"""

import numpy as np

import concourse.bass as bass
import concourse.bacc as bacc
import concourse.mybir as mybir
import concourse.tile as tile
from concourse import bass_utils

N_CORES = 8
N_PATCH = 4194304
NP_CORE = N_PATCH // N_CORES  # 524288
P = 128
NINE = 9
NPW = NP_CORE // P           # 4096 patches per partition per core
TILES = [256] * 15 + [128, 96, 32]   # sum = 4096
NT = len(TILES)
NDVE = 0          # trailing tiles on the exact all-DVE path (short drain)
WMAX = max(TILES)
H_OUT = 2048
PH2_SIZES = [512] * 8

f32 = mybir.dt.float32
f32r = mybir.dt.float32r
bf16 = mybir.dt.bfloat16
Alu = mybir.AluOpType
Act = mybir.ActivationFunctionType
X = mybir.AxisListType.X

_CACHE = {}


def _build(stub_cc=False):
    num_devices = 1 if stub_cc else N_CORES
    nc = bacc.Bacc("TRN2", target_bir_lowering=False, debug=False,
                   num_devices=num_devices)
    img = nc.dram_tensor("img", [NP_CORE, NINE], f32r, kind="ExternalInput")
    edg = nc.dram_tensor("edg", [NP_CORE, NINE], f32, kind="ExternalInput")
    idd = nc.dram_tensor("ident", [P, P], f32, kind="ExternalInput")
    out = nc.dram_tensor("out", [NP_CORE], f32, kind="ExternalOutput")

    # partition-contiguous patch layout: partition p owns patches
    # [p*NPW, (p+1)*NPW) of this core's shard; column slices select tiles.
    img_r = img.ap().rearrange("(p j) n -> p (j n)", p=P)
    edg_r = edg.ap().rearrange("(p j) n -> p (j n)", p=P)
    out_f = out.ap().rearrange("(p j) -> p j", p=P)

    with tile.TileContext(nc) as tc:
        with (
            tc.tile_pool(name="vin", bufs=5) as vpool,
            tc.tile_pool(name="ein", bufs=5) as epool,
            tc.tile_pool(name="sr", bufs=5) as spool,
            tc.tile_pool(name="sm", bufs=3) as smpool,
            tc.tile_pool(name="ph2", bufs=3) as ph2pool,
            tc.tile_pool(name="persist", bufs=1) as pers,
            tc.tile_pool(name="psum", bufs=3, space="PSUM") as psum,
            tc.tile_pool(name="psmall", bufs=1, space="PSUM") as psmall,
            tc.tile_pool(name="dram", bufs=1, space="DRAM") as dram,
        ):
            vals = pers.tile([P, NPW], f32)
            marr = pers.tile([P, NPW], bf16)
            barr = pers.tile([P, NPW], bf16)
            # per-tile accumulator columns: [SO | SU | CO | CU | SA] x NT
            acc = pers.tile([P, 5 * NT], f32)
            accSO = acc[:, 0 * NT:1 * NT]
            accSU = acc[:, 1 * NT:2 * NT]
            accCO = acc[:, 2 * NT:3 * NT]
            accCU = acc[:, 3 * NT:4 * NT]
            accSA = acc[:, 4 * NT:5 * NT]
            nbias = pers.tile([P, 1], f32)
            nc.vector.memset(nbias[:], -0.5)
            ones_col = pers.tile([P, 1], f32)
            nc.vector.memset(ones_col[:], 1.0)
            ones8 = pers.tile([8, P], f32)
            nc.vector.memset(ones8[:], 1.0)
            scbP = pers.tile([P, 2], f32)
            scr = pers.tile([P, WMAX], f32)

            # identity: f32r for the V/U reduces, bf16 for the S (sign) reduce
            idf = pers.tile([P, P], f32)
            nc.sync.dma_start(idf[:], idd.ap())
            idr = pers.tile([P, P], f32r)
            nc.scalar.activation(idr[:], idf[:], Act.Identity)
            idb = pers.tile([P, P], bf16)
            nc.scalar.activation(idb[:], idf[:], Act.Identity)

            offs = [sum(TILES[:i]) for i in range(NT)]
            st = {}   # per-tile live state

            def stage_load(i):
                w = TILES[i]
                F = w * NINE
                o9 = offs[i] * NINE
                Vt = vpool.tile([P, WMAX * NINE], f32r, tag="V")
                nc.sync.dma_start(Vt[:, :F], img_r[:, o9:o9 + F])
                Et = epool.tile([P, WMAX * NINE], f32, tag="E")
                nc.sync.dma_start(Et[:, :F], edg_r[:, o9:o9 + F])
                st[i] = {"V": Vt, "E": Et}

            def stage_a(i):
                # Act: S = sign(E-0.5) as bf16 (exact); DVE: center extract
                w = TILES[i]
                F = w * NINE
                d = st[i]
                St = spool.tile([P, WMAX * NINE], bf16, tag="S")
                nc.scalar.activation(St[:, :F], d["E"][:, :F], Act.Sign,
                                     bias=nbias[:])
                v3 = d["V"][:, :F].rearrange("p (w n) -> p w n", n=NINE)
                vsl = vals[:, offs[i]:offs[i] + w]
                nc.vector.tensor_scalar(vsl, v3[:, :, 4], 1.0, None,
                                        op0=Alu.mult, op1=Alu.add,
                                        accum_out=accSA[:, i:i + 1])
                d["S"] = St

            def stage_b(i):
                # PE: sall/g group reduces
                w = TILES[i]
                F = w * NINE
                d = st[i]
                ps = psum.tile([P, 2 * 512], f32, tag="ps")
                V3 = d["V"][:, :F].rearrange("p (w n) -> p w n", n=NINE)
                S3 = d["S"][:, :F].rearrange("p (w n) -> p w n", n=NINE)
                for j in range(NINE):
                    nc.tensor.matmul(ps[:, 0:w], idr[:], V3[:, :, j],
                                     start=(j == 0), stop=(j == NINE - 1))
                for j in range(NINE):
                    nc.tensor.matmul(ps[:, 512:512 + w], idb[:],
                                     S3[:, :, j],
                                     start=(j == 0), stop=(j == NINE - 1))
                # stage g into SBUF right away (keeps the Act hop off the
                # tail critical path; verifier allows 1 PSUM read per op)
                gsb = smpool.tile([P, WMAX], f32, tag="gsb")
                nc.scalar.activation(gsb[:, :w], ps[:, 512:512 + w],
                                     Act.Identity)
                gsq = smpool.tile([P, WMAX], f32, tag="gsq")
                nc.scalar.activation(gsq[:, :w], ps[:, 512:512 + w],
                                     Act.Square)
                d.update(sall=ps[:, 0:w], sds=ps[:, 256:256 + w],
                         gsb=gsb, gsq=gsq, ps=ps)

            def stage_c(i):
                # U = V*S in place (split DVE/Pool), then PE sds group reduce
                w = TILES[i]
                F = w * NINE
                d = st[i]
                Vt, St = d["V"], d["S"]
                # q = g*sall while sall is fresh (1 PSUM read)
                q = smpool.tile([P, WMAX], f32, tag="q")
                nc.vector.tensor_tensor(q[:, :w], d["gsb"][:, :w], d["sall"],
                                        op=Alu.mult)
                d["q"] = q
                if w >= 256:
                    H = (F // 2) // 2 * 2
                    nc.vector.tensor_tensor(Vt[:, :H], Vt[:, :H], St[:, :H],
                                            op=Alu.mult)
                    nc.gpsimd.tensor_tensor(Vt[:, H:F], Vt[:, H:F],
                                            St[:, H:F], op=Alu.mult)
                else:
                    nc.vector.tensor_tensor(Vt[:, :F], Vt[:, :F], St[:, :F],
                                            op=Alu.mult)
                U3 = Vt[:, :F].rearrange("p (w n) -> p w n", n=NINE)
                ps = d["ps"]
                for j in range(NINE):
                    nc.tensor.matmul(ps[:, 256:256 + w], idr[:],
                                     U3[:, :, j],
                                     start=(j == 0), stop=(j == NINE - 1))

            def stage_d(i):
                # DVE: classification smalls + stat accums
                w = TILES[i]
                d = st[i]
                sds = d["sds"]
                vsl = vals[:, offs[i]:offs[i] + w]
                msl = marr[:, offs[i]:offs[i] + w]
                bsl = barr[:, offs[i]:offs[i] + w]
                # o01 = (9*sds <= g*sall)
                nc.vector.scalar_tensor_tensor(bsl, sds, 9.0,
                                               d["q"][:, :w],
                                               op0=Alu.mult, op1=Alu.is_le)
                # m = (g^2 == 81); CU accum
                nc.vector.tensor_scalar(msl, d["gsq"][:, :w], 81.0, None,
                                        op0=Alu.is_equal, op1=Alu.add,
                                        accum_out=accCU[:, i:i + 1])
                # A = o01 - m/2 in place over bsl (phase 2: corr = A+m*sg/2)
                # accum gives CO' = CO - CU/2
                nc.vector.scalar_tensor_tensor(bsl, msl, -0.5, bsl,
                                               op0=Alu.mult, op1=Alu.add,
                                               accum_out=accCO[:, i:i + 1])
                # masked center sums (outputs are scratch)
                # SO' = sum(A*v) = SO - SU/2
                nc.vector.scalar_tensor_tensor(scr[:, :w], bsl, 1.0, vsl,
                                               op0=Alu.mult, op1=Alu.mult,
                                               accum_out=accSO[:, i:i + 1])
                nc.vector.scalar_tensor_tensor(scr[:, :w], msl, 1.0, vsl,
                                               op0=Alu.mult, op1=Alu.mult,
                                               accum_out=accSU[:, i:i + 1])
                del st[i]

            # Skewed software pipeline (c before a/b keeps Pool's U-half of
            # tile k-2 ahead of the k-1 rounding copy in the Pool queue).
            for k in range(NT + 3):
                if 0 <= k - 3 < NT:
                    stage_d(k - 3)
                if k < NT:
                    stage_load(k)
                if 0 <= k - 2 < NT:
                    stage_c(k - 2)
                if 0 <= k - 1 < NT:
                    stage_a(k - 1)
                    stage_b(k - 1)

            # ---- per-core stats -> [1,5] ----
            acc8 = pers.tile([P, 5], f32)
            acc5 = acc[:].rearrange("p (j i) -> p j i", j=5)
            nc.vector.tensor_reduce(acc8[:], acc5, axis=X, op=Alu.add)
            gsb = pers.tile([1, 5], f32)
            nc.gpsimd.tensor_reduce(gsb[:], acc8[:],
                                    axis=mybir.AxisListType.C, op=Alu.add)

            # ---- AllGather the [1,5] partials across cores ----
            cc_in = dram.tile([1, 5], f32)
            cc_out = dram.tile([8, 5], f32, addr_space="Shared")
            nc.sync.dma_start(cc_in[:], gsb[:])
            gt8 = pers.tile([8, 5], f32)
            if stub_cc:
                nc.vector.memset(gt8[:], 0.0)
                nc.sync.dma_start(gt8[0:1, :], cc_in[:])
            else:
                nc.gpsimd.collective_compute(
                    "AllGather", Alu.bypass,
                    replica_groups=[list(range(N_CORES))],
                    ins=[cc_in[:].opt()], outs=[cc_out[:].opt()])
                nc.sync.dma_start(gt8[:], cc_out[:])
            # sum the 8 per-core rows AND broadcast to all partitions in one
            # matmul: ones[8,P].T @ gt8[8,5] -> [P,5]
            pb = psmall.tile([P, 8], f32)
            nc.tensor.matmul(pb[:, 0:5], ones8[:], gt8[:], start=True,
                             stop=True)
            gtP = pers.tile([P, 5], f32)
            nc.vector.tensor_copy(gtP[:], pb[:, 0:5])

            # gtP cols = [SO', SU, CO', CU, SA], SO' = SO-SU/2, CO' = CO-CU/2
            # SW = SO-SU = SO'-SU/2, CW = CO-CU = CO'-CU/2,
            # SB = SA-SO = SA-SO'-SU/2, CB = N-CO
            tmp = pers.tile([P, 8], f32)
            avg = pers.tile([P, 2], f32)
            rc = pers.tile([P, 2], f32)
            nc.vector.scalar_tensor_tensor(tmp[:, 5:6], gtP[:, 3:4], -0.5,
                                           gtP[:, 2:3], op0=Alu.mult,
                                           op1=Alu.add)          # CW
            nc.vector.scalar_tensor_tensor(tmp[:, 2:3], gtP[:, 3:4], 0.5,
                                           gtP[:, 2:3], op0=Alu.mult,
                                           op1=Alu.add)          # CO
            nc.vector.tensor_scalar(tmp[:, 4:5], tmp[:, 2:3], -1.0,
                                    float(N_PATCH), op0=Alu.mult,
                                    op1=Alu.add)                 # CB
            nc.vector.reciprocal(rc[:], tmp[:, 4:6])             # [rcb, rcw]
            nc.vector.scalar_tensor_tensor(tmp[:, 1:2], gtP[:, 1:2], -0.5,
                                           gtP[:, 0:1], op0=Alu.mult,
                                           op1=Alu.add)          # SW
            nc.vector.tensor_tensor(tmp[:, 7:8], gtP[:, 4:5], gtP[:, 0:1],
                                    op=Alu.subtract)             # SA-SO'
            nc.vector.scalar_tensor_tensor(tmp[:, 0:1], gtP[:, 1:2], -0.5,
                                           tmp[:, 7:8], op0=Alu.mult,
                                           op1=Alu.add)          # SB
            nc.vector.tensor_tensor(avg[:], tmp[:, 0:2], rc[:],
                                    op=Alu.mult)                 # [avgB, avgW]
            nc.vector.tensor_tensor(scbP[:, 0:1], avg[:, 1:2], avg[:, 0:1],
                                    op=Alu.subtract)             # s
            nc.vector.tensor_tensor(tmp[:, 6:7], avg[:, 0:1], avg[:, 1:2],
                                    op=Alu.add)                  # avgB+avgW
            nc.vector.scalar_tensor_tensor(scbP[:, 1:2], tmp[:, 6:7], -0.5,
                                           scbP[:, 0:1], op0=Alu.mult,
                                           op1=Alu.mult)         # -mid*s

            # ---- phase 2: corr = A + 0.5*m*sign(v*s - mid*s) ----
            # (A = o01 - m/2 is in barr; sign==-1 exactly when w1<0)
            PH2MAX = max(PH2_SIZES)
            po = 0
            for ci, cw in enumerate(PH2_SIZES):
                sl = slice(po, po + cw)
                sg = ph2pool.tile([P, PH2MAX], bf16, tag="sg")
                nc.scalar.activation(sg[:, :cw], vals[:, sl], Act.Sign,
                                     bias=scbP[:, 1:2], scale=scbP[:, 0:1])
                h2 = ph2pool.tile([P, PH2MAX], bf16, tag="h2")
                nc.vector.scalar_tensor_tensor(h2[:, :cw], marr[:, sl], 0.5,
                                               sg[:, :cw], op0=Alu.mult,
                                               op1=Alu.mult)
                corr = ph2pool.tile([P, PH2MAX], f32, tag="corr")
                ceng = nc.vector if ci % 2 == 0 else nc.gpsimd
                ceng.tensor_tensor(corr[:, :cw], barr[:, sl],
                                   h2[:, :cw], op=Alu.add)
                nc.sync.dma_start(out_f[:, sl], corr[:, :cw])
                po += cw

    nc.compile()
    return nc


def _get_nc():
    if "nc" not in _CACHE:
        _CACHE["nc"] = _build()
    return _CACHE["nc"]


def run(image, edges_prob, gt=None, trace=False, tmpdir=None):
    nc = _get_nc()
    img = np.ascontiguousarray(np.asarray(image), dtype=np.float32)
    edg = np.ascontiguousarray(np.asarray(edges_prob), dtype=np.float32)
    img = img.reshape(N_PATCH, NINE)
    edg = edg.reshape(N_PATCH, NINE)
    ident = np.eye(P, dtype=np.float32)
    in_maps = []
    for c in range(N_CORES):
        sl = slice(c * NP_CORE, (c + 1) * NP_CORE)
        in_maps.append({"img": img[sl], "edg": edg[sl], "ident": ident})
    res = bass_utils.run_bass_kernel_spmd(
        nc, in_maps, core_ids=list(range(N_CORES)),
        trace=trace, tmpdir=tmpdir)
    shards = []
    for c in range(N_CORES):
        shards.append(res.results[c]["out"])
    full = np.concatenate(shards).reshape(H_OUT, H_OUT)
    return full, res


def kernel(image, edges_prob, gt=None, **_ignored):
    full, _ = run(image, edges_prob, gt)
    return full


def _numpy_model(image, edges_prob):
    img = np.asarray(image).reshape(N_PATCH, NINE)
    edg = np.asarray(edges_prob).reshape(N_PATCH, NINE)
    S = np.where(edg > 0.5, 1.0, -1.0).astype(np.float32)
    g = S.sum(1)
    sds = (img * S).sum(1)
    sall = img.sum(1)
    o01 = (9.0 * sds <= g * sall).astype(np.float32)
    unk = np.abs(g) == 9
    v = img[:, 4]
    mb = (~unk) & (o01 == 0.0)
    mw = (~unk) & (o01 == 1.0)
    avgB = (v * mb).sum() / max(mb.sum(), 1)
    avgW = (v * mw).sum() / max(mw.sum(), 1)
    cls = (np.abs(v - avgB) >= np.abs(v - avgW)).astype(np.float32)
    corr = np.where(unk, cls, o01)
    return corr.reshape(H_OUT, H_OUT)


# revision 53
# speedup vs baseline: 1.0429x; 1.0073x over previous
"""Trainium2 Bass kernel for nn_Predictor_67585605370461 (segment_reduce).

Per patch (N=4194304, 9 elements each):
  S = sign(E-0.5) in {-1,+1}; g = sum(S); sall = sum(V); sds = sum(V*S)
  out01 = (9*sds <= g*sall)   [equivalent to md <= ma]
  unknown m = (|g| == 9); b = o01 (o01==1 for unknowns)
Global masked means avgB/avgW of center pixels over the two known classes;
  corr = o01 - m*(w1 < 0), w1 = v*s - mid*s, s = avgW-avgB, mid = (avgB+avgW)/2

Engine split (all big 9-wide group reductions on the otherwise-idle PE via
identity-matmul PSUM accumulation in fp32r; ~2^-14 rounding, validated):
  Act:  Vr = round_f32r(V), S = sign(E-0.5) as f32r
  DVE:  U = Vr*S (f32r), center extract, classification smalls + stat accums
  PE:   sall/sds/g = 9-slice Id-matmul accumulations (fp32r, 1 cyc/col)
  Pool: phase-2 helper chunks
Cross-core: one [1,5] -> [8,5] AllGather of per-core stat partials (cheaper
than AllReduce), then a tiny PE ones-matmul partition reduce.

Sharding: data-parallel over the patch axis, 524288 patches per core.
"""

import numpy as np

import concourse.bass as bass
import concourse.bacc as bacc
import concourse.mybir as mybir
import concourse.tile as tile
from concourse import bass_utils

N_CORES = 8
N_PATCH = 4194304
NP_CORE = N_PATCH // N_CORES  # 524288
P = 128
NINE = 9
NPW = NP_CORE // P           # 4096 patches per partition per core
TILES = [256] * 15 + [128, 96, 32]   # sum = 4096
NT = len(TILES)
NDVE = 0          # trailing tiles on the exact all-DVE path (short drain)
WMAX = max(TILES)
H_OUT = 2048
PH2_SIZES = [512] * 8

f32 = mybir.dt.float32
f32r = mybir.dt.float32r
bf16 = mybir.dt.bfloat16
Alu = mybir.AluOpType
Act = mybir.ActivationFunctionType
X = mybir.AxisListType.X

_CACHE = {}


def _build(stub_cc=False):
    num_devices = 1 if stub_cc else N_CORES
    nc = bacc.Bacc("TRN2", target_bir_lowering=False, debug=False,
                   num_devices=num_devices)
    img = nc.dram_tensor("img", [NP_CORE, NINE], f32r, kind="ExternalInput")
    edg = nc.dram_tensor("edg", [NP_CORE, NINE], f32, kind="ExternalInput")
    idd = nc.dram_tensor("ident", [P, P], f32, kind="ExternalInput")
    out = nc.dram_tensor("out", [NP_CORE], f32, kind="ExternalOutput")

    # partition-contiguous patch layout: partition p owns patches
    # [p*NPW, (p+1)*NPW) of this core's shard; column slices select tiles.
    img_r = img.ap().rearrange("(p j) n -> p (j n)", p=P)
    edg_r = edg.ap().rearrange("(p j) n -> p (j n)", p=P)
    out_f = out.ap().rearrange("(p j) -> p j", p=P)

    with tile.TileContext(nc) as tc:
        with (
            tc.tile_pool(name="vin", bufs=5) as vpool,
            tc.tile_pool(name="ein", bufs=5) as epool,
            tc.tile_pool(name="sr", bufs=5) as spool,
            tc.tile_pool(name="sm", bufs=3) as smpool,
            tc.tile_pool(name="ph2", bufs=3) as ph2pool,
            tc.tile_pool(name="persist", bufs=1) as pers,
            tc.tile_pool(name="psum", bufs=3, space="PSUM") as psum,
            tc.tile_pool(name="psmall", bufs=1, space="PSUM") as psmall,
            tc.tile_pool(name="dram", bufs=1, space="DRAM") as dram,
        ):
            vals = pers.tile([P, NPW], f32)
            marr = pers.tile([P, NPW], bf16)
            barr = pers.tile([P, NPW], bf16)
            # per-tile accumulator columns: [SO | SU | CO | CU | SA] x NT
            acc = pers.tile([P, 5 * NT], f32)
            accSO = acc[:, 0 * NT:1 * NT]
            accSU = acc[:, 1 * NT:2 * NT]
            accCO = acc[:, 2 * NT:3 * NT]
            accCU = acc[:, 3 * NT:4 * NT]
            accSA = acc[:, 4 * NT:5 * NT]
            nbias = pers.tile([P, 1], f32)
            nc.vector.memset(nbias[:], -0.5)
            ones_col = pers.tile([P, 1], f32)
            nc.vector.memset(ones_col[:], 1.0)
            ones8 = pers.tile([8, P], f32)
            nc.vector.memset(ones8[:], 1.0)
            scbP = pers.tile([P, 2], f32)
            scr = pers.tile([P, WMAX], f32)

            # identity: f32r for the V/U reduces, bf16 for the S (sign) reduce
            idf = pers.tile([P, P], f32)
            nc.sync.dma_start(idf[:], idd.ap())
            idr = pers.tile([P, P], f32r)
            nc.scalar.activation(idr[:], idf[:], Act.Identity)
            idb = pers.tile([P, P], bf16)
            nc.scalar.activation(idb[:], idf[:], Act.Identity)

            offs = [sum(TILES[:i]) for i in range(NT)]
            st = {}   # per-tile live state

            def stage_load(i):
                w = TILES[i]
                F = w * NINE
                o9 = offs[i] * NINE
                Vt = vpool.tile([P, WMAX * NINE], f32r, tag="V")
                nc.sync.dma_start(Vt[:, :F], img_r[:, o9:o9 + F])
                Et = epool.tile([P, WMAX * NINE], f32, tag="E")
                nc.sync.dma_start(Et[:, :F], edg_r[:, o9:o9 + F])
                st[i] = {"V": Vt, "E": Et}

            def stage_a(i):
                # Act: S = sign(E-0.5) as bf16 (exact); DVE: center extract
                w = TILES[i]
                F = w * NINE
                d = st[i]
                St = spool.tile([P, WMAX * NINE], bf16, tag="S")
                nc.scalar.activation(St[:, :F], d["E"][:, :F], Act.Sign,
                                     bias=nbias[:])
                v3 = d["V"][:, :F].rearrange("p (w n) -> p w n", n=NINE)
                vsl = vals[:, offs[i]:offs[i] + w]
                nc.scalar.activation(vsl, v3[:, :, 4], Act.Identity,
                                     accum_out=accSA[:, i:i + 1])
                d["S"] = St

            def stage_b(i):
                # PE: sall/g group reduces (last tile: all-DVE, no PSUM)
                w = TILES[i]
                F = w * NINE
                d = st[i]
                if i == NT - 1:
                    red = smpool.tile([P, 3 * 32], f32, tag="red")
                    V3 = d["V"][:, :F].rearrange("p (w n) -> p w n", n=NINE)
                    S3 = d["S"][:, :F].rearrange("p (w n) -> p w n", n=NINE)
                    nc.vector.tensor_reduce(red[:, 0:w], V3, axis=X,
                                            op=Alu.add)
                    nc.vector.tensor_reduce(red[:, 64:64 + w], S3, axis=X,
                                            op=Alu.add)
                    d.update(sall=red[:, 0:w], sds=red[:, 32:32 + w],
                             g=red[:, 64:64 + w], red=red)
                    return
                ps = psum.tile([P, 2 * 512], f32, tag="ps")
                V3 = d["V"][:, :F].rearrange("p (w n) -> p w n", n=NINE)
                S3 = d["S"][:, :F].rearrange("p (w n) -> p w n", n=NINE)
                for j in range(NINE):
                    nc.tensor.matmul(ps[:, 0:w], idr[:], V3[:, :, j],
                                     start=(j == 0), stop=(j == NINE - 1))
                for j in range(NINE):
                    nc.tensor.matmul(ps[:, 512:512 + w], idb[:],
                                     S3[:, :, j],
                                     start=(j == 0), stop=(j == NINE - 1))
                # stage g into SBUF right away (keeps the Act hop off the
                # tail critical path; verifier allows 1 PSUM read per op)
                gsb = smpool.tile([P, WMAX], f32, tag="gsb")
                nc.scalar.activation(gsb[:, :w], ps[:, 512:512 + w],
                                     Act.Identity)
                gsq = smpool.tile([P, WMAX], f32, tag="gsq")
                nc.scalar.activation(gsq[:, :w], ps[:, 512:512 + w],
                                     Act.Square)
                d.update(sall=ps[:, 0:w], sds=ps[:, 256:256 + w],
                         gsb=gsb, gsq=gsq, ps=ps)

            def stage_c(i):
                # U = V*S in place (split DVE/Pool), then PE sds group reduce
                w = TILES[i]
                F = w * NINE
                d = st[i]
                Vt, St = d["V"], d["S"]
                if i == NT - 1:
                    nc.vector.tensor_tensor(Vt[:, :F], Vt[:, :F], St[:, :F],
                                            op=Alu.mult)
                    U3 = Vt[:, :F].rearrange("p (w n) -> p w n", n=NINE)
                    nc.vector.tensor_reduce(d["red"][:, 32:32 + w], U3,
                                            axis=X, op=Alu.add)
                    q = smpool.tile([P, WMAX], f32, tag="q")
                    nc.vector.tensor_tensor(q[:, :w], d["g"], d["sall"],
                                            op=Alu.mult)
                    d["q"] = q
                    return
                # q = g*sall while sall is fresh (1 PSUM read)
                q = smpool.tile([P, WMAX], f32, tag="q")
                nc.vector.tensor_tensor(q[:, :w], d["gsb"][:, :w], d["sall"],
                                        op=Alu.mult)
                d["q"] = q
                if w >= 256:
                    H = (F // 2) // 2 * 2
                    nc.vector.tensor_tensor(Vt[:, :H], Vt[:, :H], St[:, :H],
                                            op=Alu.mult)
                    nc.gpsimd.tensor_tensor(Vt[:, H:F], Vt[:, H:F],
                                            St[:, H:F], op=Alu.mult)
                else:
                    # tail tiles: full product on Pool to drain DVE faster
                    nc.gpsimd.tensor_tensor(Vt[:, :F], Vt[:, :F], St[:, :F],
                                            op=Alu.mult)
                U3 = Vt[:, :F].rearrange("p (w n) -> p w n", n=NINE)
                ps = d["ps"]
                for j in range(NINE):
                    nc.tensor.matmul(ps[:, 256:256 + w], idr[:],
                                     U3[:, :, j],
                                     start=(j == 0), stop=(j == NINE - 1))

            def stage_d(i):
                # DVE: classification smalls + stat accums
                w = TILES[i]
                d = st[i]
                sds = d["sds"]
                vsl = vals[:, offs[i]:offs[i] + w]
                msl = marr[:, offs[i]:offs[i] + w]
                bsl = barr[:, offs[i]:offs[i] + w]
                # o01 = (9*sds <= g*sall)
                nc.vector.scalar_tensor_tensor(bsl, sds, 9.0,
                                               d["q"][:, :w],
                                               op0=Alu.mult, op1=Alu.is_le)
                if i == NT - 1:
                    # all-DVE tail: gsq = g*g, then m = (gsq == 81)
                    gq = smpool.tile([P, WMAX], f32, tag="gq")
                    nc.vector.tensor_tensor(gq[:, :w], d["g"], d["g"],
                                            op=Alu.mult)
                    nc.vector.tensor_scalar(msl, gq[:, :w], 81.0, None,
                                            op0=Alu.is_equal, op1=Alu.add,
                                            accum_out=accCU[:, i:i + 1])
                else:
                    # m = (g^2 == 81); CU accum
                    nc.vector.tensor_scalar(msl, d["gsq"][:, :w], 81.0, None,
                                            op0=Alu.is_equal, op1=Alu.add,
                                            accum_out=accCU[:, i:i + 1])
                # A = o01 - m/2 in place over bsl (phase 2: corr = A+m*sg/2)
                # accum gives CO' = CO - CU/2
                nc.vector.scalar_tensor_tensor(bsl, msl, -0.5, bsl,
                                               op0=Alu.mult, op1=Alu.add,
                                               accum_out=accCO[:, i:i + 1])
                # masked center sums (outputs are scratch)
                # SO' = sum(A*v) = SO - SU/2
                nc.vector.scalar_tensor_tensor(scr[:, :w], bsl, 1.0, vsl,
                                               op0=Alu.mult, op1=Alu.mult,
                                               accum_out=accSO[:, i:i + 1])
                nc.vector.scalar_tensor_tensor(scr[:, :w], msl, 1.0, vsl,
                                               op0=Alu.mult, op1=Alu.mult,
                                               accum_out=accSU[:, i:i + 1])
                del st[i]

            # Skewed software pipeline.
            for k in range(NT + 3):
                if 0 <= k - 3 < NT:
                    stage_d(k - 3)
                if k < NT:
                    stage_load(k)
                if 0 <= k - 2 < NT:
                    stage_c(k - 2)
                if 0 <= k - 1 < NT:
                    stage_a(k - 1)
                    stage_b(k - 1)

            # ---- per-core stats -> [1,5] ----
            acc8 = pers.tile([P, 5], f32)
            acc5 = acc[:].rearrange("p (j i) -> p j i", j=5)
            nc.vector.tensor_reduce(acc8[:], acc5, axis=X, op=Alu.add)
            gsb = pers.tile([1, 5], f32)
            nc.gpsimd.tensor_reduce(gsb[:], acc8[:],
                                    axis=mybir.AxisListType.C, op=Alu.add)

            # ---- AllGather the [1,5] partials across cores ----
            cc_in = dram.tile([1, 5], f32)
            cc_out = dram.tile([8, 5], f32, addr_space="Shared")
            nc.sync.dma_start(cc_in[:], gsb[:])
            gt8 = pers.tile([8, 5], f32)
            if stub_cc:
                nc.vector.memset(gt8[:], 0.0)
                nc.sync.dma_start(gt8[0:1, :], cc_in[:])
            else:
                nc.gpsimd.collective_compute(
                    "AllGather", Alu.bypass,
                    replica_groups=[list(range(N_CORES))],
                    ins=[cc_in[:].opt()], outs=[cc_out[:].opt()])
                nc.sync.dma_start(gt8[:], cc_out[:])
            # sum the 8 per-core rows AND broadcast to all partitions in one
            # matmul: ones[8,P].T @ gt8[8,5] -> [P,5]
            pb = psmall.tile([P, 8], f32)
            nc.tensor.matmul(pb[:, 0:5], ones8[:], gt8[:], start=True,
                             stop=True)
            gtP = pers.tile([P, 5], f32)
            nc.vector.tensor_copy(gtP[:], pb[:, 0:5])

            # gtP cols = [SO', SU, CO', CU, SA], SO' = SO-SU/2, CO' = CO-CU/2
            # SW = SO-SU = SO'-SU/2, CW = CO-CU = CO'-CU/2,
            # SB = SA-SO = SA-SO'-SU/2, CB = N-CO
            tmp = pers.tile([P, 8], f32)
            avg = pers.tile([P, 2], f32)
            rc = pers.tile([P, 2], f32)
            nc.vector.scalar_tensor_tensor(tmp[:, 5:6], gtP[:, 3:4], -0.5,
                                           gtP[:, 2:3], op0=Alu.mult,
                                           op1=Alu.add)          # CW
            nc.vector.scalar_tensor_tensor(tmp[:, 2:3], gtP[:, 3:4], 0.5,
                                           gtP[:, 2:3], op0=Alu.mult,
                                           op1=Alu.add)          # CO
            nc.vector.tensor_scalar(tmp[:, 4:5], tmp[:, 2:3], -1.0,
                                    float(N_PATCH), op0=Alu.mult,
                                    op1=Alu.add)                 # CB
            nc.vector.reciprocal(rc[:], tmp[:, 4:6])             # [rcb, rcw]
            nc.vector.scalar_tensor_tensor(tmp[:, 1:2], gtP[:, 1:2], -0.5,
                                           gtP[:, 0:1], op0=Alu.mult,
                                           op1=Alu.add)          # SW
            nc.vector.tensor_tensor(tmp[:, 7:8], gtP[:, 4:5], gtP[:, 0:1],
                                    op=Alu.subtract)             # SA-SO'
            nc.vector.scalar_tensor_tensor(tmp[:, 0:1], gtP[:, 1:2], -0.5,
                                           tmp[:, 7:8], op0=Alu.mult,
                                           op1=Alu.add)          # SB
            nc.vector.tensor_tensor(avg[:], tmp[:, 0:2], rc[:],
                                    op=Alu.mult)                 # [avgB, avgW]
            nc.vector.tensor_tensor(scbP[:, 0:1], avg[:, 1:2], avg[:, 0:1],
                                    op=Alu.subtract)             # s
            nc.vector.tensor_tensor(tmp[:, 6:7], avg[:, 0:1], avg[:, 1:2],
                                    op=Alu.add)                  # avgB+avgW
            nc.vector.scalar_tensor_tensor(scbP[:, 1:2], tmp[:, 6:7], -0.5,
                                           scbP[:, 0:1], op0=Alu.mult,
                                           op1=Alu.mult)         # -mid*s

            # ---- phase 2: corr = A + 0.5*m*sign(v*s - mid*s) ----
            # (A = o01 - m/2 is in barr; sign==-1 exactly when w1<0)
            PH2MAX = max(PH2_SIZES)
            po = 0
            for ci, cw in enumerate(PH2_SIZES):
                sl = slice(po, po + cw)
                sg = ph2pool.tile([P, PH2MAX], bf16, tag="sg")
                nc.scalar.activation(sg[:, :cw], vals[:, sl], Act.Sign,
                                     bias=scbP[:, 1:2], scale=scbP[:, 0:1])
                # h2 on DVE (Pool lacks STT); Pool takes corr on some chunks
                h2 = ph2pool.tile([P, PH2MAX], bf16, tag="h2")
                nc.vector.scalar_tensor_tensor(h2[:, :cw], marr[:, sl], 0.5,
                                               sg[:, :cw], op0=Alu.mult,
                                               op1=Alu.mult)
                corr = ph2pool.tile([P, PH2MAX], f32, tag="corr")
                ceng = nc.gpsimd if ci in (1, 3, 5) else nc.vector
                ceng.tensor_tensor(corr[:, :cw], barr[:, sl],
                                   h2[:, :cw], op=Alu.add)
                nc.sync.dma_start(out_f[:, sl], corr[:, :cw])
                po += cw

    nc.compile()
    return nc


def _get_nc():
    if "nc" not in _CACHE:
        _CACHE["nc"] = _build()
    return _CACHE["nc"]


def run(image, edges_prob, gt=None, trace=False, tmpdir=None):
    nc = _get_nc()
    img = np.ascontiguousarray(np.asarray(image), dtype=np.float32)
    edg = np.ascontiguousarray(np.asarray(edges_prob), dtype=np.float32)
    img = img.reshape(N_PATCH, NINE)
    edg = edg.reshape(N_PATCH, NINE)
    ident = np.eye(P, dtype=np.float32)
    in_maps = []
    for c in range(N_CORES):
        sl = slice(c * NP_CORE, (c + 1) * NP_CORE)
        in_maps.append({"img": img[sl], "edg": edg[sl], "ident": ident})
    res = bass_utils.run_bass_kernel_spmd(
        nc, in_maps, core_ids=list(range(N_CORES)),
        trace=trace, tmpdir=tmpdir)
    shards = []
    for c in range(N_CORES):
        shards.append(res.results[c]["out"])
    full = np.concatenate(shards).reshape(H_OUT, H_OUT)
    return full, res


def kernel(image, edges_prob, gt=None, **_ignored):
    full, _ = run(image, edges_prob, gt)
    return full


def _numpy_model(image, edges_prob):
    img = np.asarray(image).reshape(N_PATCH, NINE)
    edg = np.asarray(edges_prob).reshape(N_PATCH, NINE)
    S = np.where(edg > 0.5, 1.0, -1.0).astype(np.float32)
    g = S.sum(1)
    sds = (img * S).sum(1)
    sall = img.sum(1)
    o01 = (9.0 * sds <= g * sall).astype(np.float32)
    unk = np.abs(g) == 9
    v = img[:, 4]
    mb = (~unk) & (o01 == 0.0)
    mw = (~unk) & (o01 == 1.0)
    avgB = (v * mb).sum() / max(mb.sum(), 1)
    avgW = (v * mw).sum() / max(mw.sum(), 1)
    cls = (np.abs(v - avgB) >= np.abs(v - avgW)).astype(np.float32)
    corr = np.where(unk, cls, o01)
    return corr.reshape(H_OUT, H_OUT)
